# Initial kernel scaffold
#
"""ChirpletKANLinear forward on 8 Trainium2 NeuronCores.

Math (per reference):
    base_out[b,o]  = sum_i silu(x[b,i]) * BW[o,i]
    xs             = (x[b,i] - T[o,i]) / S[o,i]
    chirp[b,o,i]   = cos(2*pi*F[o,i]*xs) * exp(-0.5*xs^2)
    out[b,o]       = base_out + sum_i chirp * CW[o,i] + bias[o]

Sharding: out-features across the 8 cores (64 each), full batch per core.

On-device formulation (per core, per (o, i-chunk) tile of [128 i, 1024 b]):
    DVE: m = u2*x + v2          u2 = F/S, v2 = 0.25 - F*T/S + 16.5
    DVE: d = (m mod 1) - 0.5    in [-0.5, 0.5)
    ACT: sinv = Sin(2*pi*d)     = -cos(2*pi*F*xs)
    ACT: derf = Derivative_Erf(w*x + p) = (2/sqrt(pi))*exp(-0.5*xs^2)
                                w = 1/(sqrt(2)*S), p = -T/(sqrt(2)*S)
    DVE: g = sinv * derf        (bf16)
    PE : psum[o, b] += lhsT_col(o)^T @ g     lhsT_col = -sqrt(pi)/2 * CW column
The -sqrt(pi)/2 factor absorbs both ACT constant factors and the sign.
Base path: Silu on ACT once, fp32 matmuls into the same PSUM accumulator.
"""

import math

import numpy as np
import ml_dtypes

import concourse.bass as bass
import concourse.tile as tile
import concourse.mybir as mybir
from concourse.bass_utils import run_bass_kernel_spmd

B, IN, OUT = 1024, 512, 512
NCORES = 8
OSH = OUT // NCORES          # 64 out features per core
NCH = IN // 128              # 4 contraction chunks of 128 partitions
G = 32                       # tiles per ACT table-set phase
HALF = B // 2                # 512, one PSUM bank of fp32 per matmul

F32 = mybir.dt.float32
F16 = mybir.dt.float16
BF16 = mybir.dt.bfloat16
AF = mybir.ActivationFunctionType
ALU = mybir.AluOpType
TWO_PI = 2.0 * math.pi

TRACE = False
LAST_RESULT = None

_nc_cache = None


def _build_nc():
    nc = bass.Bass("TRN2", target_bir_lowering=False, debug=False,
                   num_devices=NCORES)

    xT_d = nc.dram_tensor("xT", [NCH, 128, B], F32, kind="ExternalInput")
    sinm_d = nc.dram_tensor("sinm", [128, NCH, OSH], F32, kind="ExternalInput")
    sinb_d = nc.dram_tensor("sinb", [128, NCH, OSH], F32, kind="ExternalInput")
    gm_d = nc.dram_tensor("gm", [128, NCH, OSH], F32, kind="ExternalInput")
    gb_d = nc.dram_tensor("gb", [128, NCH, OSH], F32, kind="ExternalInput")
    cwsp_d = nc.dram_tensor("cwsp", [128, NCH, OSH, OSH], BF16, kind="ExternalInput")
    bw_d = nc.dram_tensor("bw", [128, NCH, OSH], F32, kind="ExternalInput")
    bias_d = nc.dram_tensor("biasv", [OSH, 1], F32, kind="ExternalOutput" == "x" and None or "ExternalInput")
    out_d = nc.dram_tensor("out", [OSH, B], F32, kind="ExternalOutput")

    with tile.TileContext(nc) as tc:
        with (
            tc.tile_pool(name="singles", bufs=1) as singles,
            tc.tile_pool(name="mpool", bufs=4) as mpool,
            tc.tile_pool(name="dpool", bufs=6) as dpool,
            tc.tile_pool(name="cospool", bufs=G + 2) as cospool,
            tc.tile_pool(name="gausspool", bufs=3) as gausspool,
            tc.tile_pool(name="gpool", bufs=3) as gpool,
            tc.tile_pool(name="psum", bufs=1, space=bass.MemorySpace.PSUM) as psump,
        ):
            xT_sb = singles.tile([128, NCH, B], F32)
            for c in range(NCH):
                nc.sync.dma_start(xT_sb[:, c, :], xT_d[c])
            sinm_sb = singles.tile([128, NCH, OSH], F32)
            nc.sync.dma_start(sinm_sb[:], sinm_d[:])
            sinb_sb = singles.tile([128, NCH, OSH], F32)
            nc.sync.dma_start(sinb_sb[:], sinb_d[:])
            gm_sb = singles.tile([128, NCH, OSH], F32)
            nc.sync.dma_start(gm_sb[:], gm_d[:])
            gb_sb = singles.tile([128, NCH, OSH], F32)
            nc.sync.dma_start(gb_sb[:], gb_d[:])
            cwsp_sb = singles.tile([128, NCH, OSH, OSH], BF16)
            nc.sync.dma_start(cwsp_sb[:], cwsp_d[:])
            bw_sb = singles.tile([128, NCH, OSH], F32)
            nc.sync.dma_start(bw_sb[:], bw_d[:])
            bias_sb = singles.tile([OSH, 1], F32)
            nc.sync.dma_start(bias_sb[:], bias_d[:])

            psum_acc = psump.tile([OSH, B], F32)

            # Silu first: same ACT table set as Sin (silu_and_others).
            silu_sb = singles.tile([128, NCH, B], F32)
            for c in range(NCH):
                nc.scalar.activation(silu_sb[:, c, :], xT_sb[:, c, :], AF.Silu)

            # Base-path matmuls open the PSUM accumulation groups.
            for h in range(2):
                for c in range(NCH):
                    nc.tensor.matmul(
                        psum_acc[:, h * HALF:(h + 1) * HALF],
                        bw_sb[:, c, :],
                        silu_sb[:, c, h * HALF:(h + 1) * HALF],
                        start=(c == 0), stop=False,
                        skip_group_check=True,
                    )

            tiles = [(o, c) for o in range(OSH) for c in range(NCH)]
            ntiles = len(tiles)
            for g0 in range(0, ntiles, G):
                group = tiles[g0:g0 + G]
                cos_tiles = []
                # --- Sin phase (table set: silu_and_others) ---
                for (o, c) in group:
                    m_t = mpool.tile([128, B], F32)
                    nc.vector.tensor_scalar(
                        m_t, xT_sb[:, c, :],
                        sinm_sb[:, c, o:o + 1], sinb_sb[:, c, o:o + 1],
                        ALU.mult, ALU.add)
                    d_t = dpool.tile([128, B], F16)
                    nc.vector.tensor_scalar(
                        d_t, m_t, 1.0, 0.5, ALU.mod, ALU.subtract)
                    c_t = cospool.tile([128, B], BF16)
                    nc.scalar.activation(c_t, d_t, AF.Sin, bias=0.0, scale=TWO_PI)
                    cos_tiles.append(c_t)
                # --- Derivative_Erf phase (table set: erf_derivative) ---
                for idx, (o, c) in enumerate(group):
                    ga_t = gausspool.tile([128, B], BF16)
                    nc.scalar.activation(
                        ga_t, xT_sb[:, c, :], AF.Derivative_Erf,
                        bias=gb_sb[:, c, o:o + 1], scale=gm_sb[:, c, o:o + 1])
                    g_t = gpool.tile([128, B], BF16)
                    nc.vector.tensor_tensor(g_t, cos_tiles[idx], ga_t, ALU.mult)
                    last = (g0 + idx == ntiles - 1)
                    for h in range(2):
                        nc.tensor.matmul(
                            psum_acc[:, h * HALF:(h + 1) * HALF],
                            cwsp_sb[:, c, o, :],
                            g_t[:, h * HALF:(h + 1) * HALF],
                            start=False, stop=last,
                            skip_group_check=True,
                        )

            out_sb = singles.tile([OSH, B], F32)
            nc.scalar.activation(out_sb, psum_acc, AF.Identity,
                                 bias=bias_sb[:, 0:1], scale=1.0)
            nc.sync.dma_start(out_d[:], out_sb[:])

    return nc


def _plane(a):
    """[OSH, IN] param -> [128 part, NCH, OSH] per-partition plane."""
    return np.ascontiguousarray(
        a.reshape(OSH, NCH, 128).transpose(2, 1, 0).astype(np.float32))


def _host_prep(inp):
    x = inp["x"]
    xT = np.ascontiguousarray(x.T.reshape(NCH, 128, B).astype(np.float32))
    maps = []
    for k in range(NCORES):
        sl = slice(k * OSH, (k + 1) * OSH)
        fk = inp["frequency"][sl]
        sk = inp["scale"][sl]
        tk = inp["translation"][sl]
        cwk = inp["chirplet_weights"][sl]
        bwk = inp["base_weight"][sl]
        u2 = fk / sk
        v2 = 0.25 - fk * tk / sk + 16.5   # +16.5 keeps m positive for mod
        w = 1.0 / (math.sqrt(2.0) * sk)
        p = -tk / (math.sqrt(2.0) * sk)
        lv = _plane((-math.sqrt(math.pi) / 2.0) * cwk)   # [128, NCH, OSH]
        cwsp = np.zeros((128, NCH, OSH, OSH), dtype=np.float32)
        cwsp[:, :, np.arange(OSH), np.arange(OSH)] = lv
        maps.append({
            "xT": xT,
            "sinm": _plane(u2),
            "sinb": _plane(v2),
            "gm": _plane(w),
            "gb": _plane(p),
            "cwsp": cwsp.astype(ml_dtypes.bfloat16),
            "bw": _plane(bwk),
            "biasv": np.ascontiguousarray(
                inp["bias"][sl].reshape(OSH, 1).astype(np.float32)),
        })
    return maps


def kernel(**inputs):
    global _nc_cache, LAST_RESULT
    np_in = {k: np.asarray(v, dtype=np.float32) for k, v in inputs.items()}
    if _nc_cache is None:
        _nc_cache = _build_nc()
    in_maps = _host_prep(np_in)
    res = run_bass_kernel_spmd(
        _nc_cache, in_maps, core_ids=list(range(NCORES)), trace=TRACE)
    LAST_RESULT = res
    shards = [r["out"] for r in res.results]          # each [OSH, B]
    full = np.concatenate(shards, axis=0)             # [OUT, B]
    return np.ascontiguousarray(full.T)               # [B, OUT] fp32


# revision 16
# speedup vs baseline: 1.0479x; 1.0479x over previous
"""ChirpletKANLinear forward on 8 Trainium2 NeuronCores.

Math (per reference):
    base_out[b,o]  = sum_i silu(x[b,i]) * BW[o,i]
    xs             = (x[b,i] - T[o,i]) / S[o,i]
    chirp[b,o,i]   = cos(2*pi*F[o,i]*xs) * exp(-0.5*xs^2)
    out[b,o]       = base_out + sum_i chirp * CW[o,i] + bias[o]

Sharding: out-features across the 8 cores (64 each), full batch per core.

On-device formulation (per core, per (o, i-chunk) tile of [128 i, 1024 b]):
    DVE: mf = int32(65536*(u2*x + v2))   u2 = F/S, v2 = 0.25 - F*T/S
    DVE: fr = (mf << 16) >> 16           low 16 bits, sign-extended =
                                         frac(m) wrapped to [-0.5, 0.5) turns
    ACT: sinv = Sin(fr * 2*pi/65536)     = cos(2*pi*F*xs)  (quarter-turn fold)
    ACT: derf = Derivative_Erf(w*x + p) = (2/sqrt(pi))*exp(-0.5*xs^2)
                                w = 1/(sqrt(2)*S), p = -T/(sqrt(2)*S)
    DVE: g = sinv * derf        (bf16)
    PE : psum[o, b] += lhsT_col(o)^T @ g     lhsT_col = sqrt(pi)/2 * CW column
The sqrt(pi)/2 factor absorbs the Derivative_Erf constant.
Base path: Silu on ACT once, fp32 matmuls into the same PSUM accumulator.
"""

import math

import numpy as np
import ml_dtypes

import concourse.bass as bass
import concourse.bacc as bacc
import concourse.tile as tile
import concourse.mybir as mybir
from concourse.bass_utils import run_bass_kernel_spmd

B, IN, OUT = 1024, 512, 512
NCORES = 8
OSH = OUT // NCORES          # 64 out features per core
NCH = IN // 128              # 4 contraction chunks of 128 partitions
G = 32                       # tiles per ACT table-set phase
HALF = B // 2                # 512, one PSUM bank of fp32 per matmul

F32 = mybir.dt.float32
F16 = mybir.dt.float16
I32 = mybir.dt.int32
BF16 = mybir.dt.bfloat16
AF = mybir.ActivationFunctionType
ALU = mybir.AluOpType
TWO_PI = 2.0 * math.pi

TRACE = False
LAST_RESULT = None

_nc_cache = None


def _build_nc(loop_r=None):
    nc = bacc.Bacc("TRN2", target_bir_lowering=False, debug=False,
                   num_devices=NCORES)

    xT_d = nc.dram_tensor("xT", [NCH, 128, B], F32, kind="ExternalInput")
    # [p, c, j, o]: j = 0 sin-scale, 1 sin-bias, 2 gauss-scale, 3 gauss-bias,
    # 4 base-weight lhsT column
    pf32_d = nc.dram_tensor("pf32", [128, NCH, 5, OSH], F32, kind="ExternalInput")
    cwsp_d = nc.dram_tensor("cwsp", [128, NCH, OSH, OSH], BF16, kind="ExternalInput")
    bias_d = nc.dram_tensor("biasv", [OSH, 1], F32, kind="ExternalInput")
    out_d = nc.dram_tensor("out", [OSH, B], F32, kind="ExternalOutput")

    with tile.TileContext(nc) as tc:
        with (
            tc.tile_pool(name="singles", bufs=1) as singles,
            tc.tile_pool(name="mpool", bufs=4) as mpool,
            tc.tile_pool(name="dpool", bufs=6) as dpool,
            tc.tile_pool(name="cospool", bufs=G + 2) as cospool,
            tc.tile_pool(name="gausspool", bufs=3) as gausspool,
            tc.tile_pool(name="gpool", bufs=3) as gpool,
            tc.tile_pool(name="psum", bufs=1, space=bass.MemorySpace.PSUM) as psump,
        ):
            xT_sb = singles.tile([128, NCH, B], F32)
            nc.sync.dma_start(xT_sb[:], xT_d.ap().rearrange("c p b -> p c b"))
            pf32_sb = singles.tile([128, NCH, 5, OSH], F32)
            nc.sync.dma_start(pf32_sb[:], pf32_d[:])
            cwsp_sb = singles.tile([128, NCH, OSH, OSH], BF16)
            nc.sync.dma_start(cwsp_sb[:], cwsp_d[:])
            bias_sb = singles.tile([OSH, 1], F32)
            nc.sync.dma_start(bias_sb[:], bias_d[:])

            psum_acc = psump.tile([OSH, B], F32)

            def compute_body():
                # Silu first: same ACT table set as Sin (silu_and_others).
                silu_sb = singles.tile([128, NCH, B], F32, tag="silu_sb")
                for c in range(NCH):
                    nc.scalar.activation(silu_sb[:, c, :], xT_sb[:, c, :],
                                         AF.Silu)

                # Base-path matmuls open the PSUM accumulation groups.
                for h in range(2):
                    for c in range(NCH):
                        nc.tensor.matmul(
                            psum_acc[:, h * HALF:(h + 1) * HALF],
                            pf32_sb[:, c, 4, :],
                            silu_sb[:, c, h * HALF:(h + 1) * HALF],
                            start=(c == 0), stop=False,
                            skip_group_check=True,
                        )

                tiles_l = [(o, c) for o in range(OSH) for c in range(NCH)]
                ntiles = len(tiles_l)
                for g0 in range(0, ntiles, G):
                    group = tiles_l[g0:g0 + G]
                    cos_tiles = []
                    # --- Sin phase (table set: silu_and_others) ---
                    for (o, c) in group:
                        m_t = mpool.tile([128, B], I32, tag="m_t")
                        nc.vector.tensor_scalar(
                            m_t, xT_sb[:, c, :],
                            pf32_sb[:, c, 0, o:o + 1], pf32_sb[:, c, 1, o:o + 1],
                            ALU.mult, ALU.add)
                        d_t = dpool.tile([128, B], I32, tag="d_t")
                        nc.vector.tensor_scalar(
                            d_t, m_t, 16, 16,
                            ALU.arith_shift_left, ALU.arith_shift_right)
                        c_t = cospool.tile([128, B], BF16, tag="c_t")
                        nc.scalar.activation(c_t, d_t, AF.Sin, bias=0.0,
                                             scale=TWO_PI / 65536.0)
                        cos_tiles.append(c_t)
                    # --- Derivative_Erf phase (table set: erf_derivative) ---
                    for idx, (o, c) in enumerate(group):
                        ga_t = gausspool.tile([128, B], BF16, tag="ga_t")
                        nc.scalar.activation(
                            ga_t, xT_sb[:, c, :], AF.Derivative_Erf,
                            bias=pf32_sb[:, c, 3, o:o + 1],
                            scale=pf32_sb[:, c, 2, o:o + 1])
                        g_t = gpool.tile([128, B], BF16, tag="g_t")
                        nc.vector.tensor_tensor(g_t, cos_tiles[idx], ga_t,
                                                ALU.mult)
                        last = (g0 + idx == ntiles - 1)
                        for h in range(2):
                            nc.tensor.matmul(
                                psum_acc[:, h * HALF:(h + 1) * HALF],
                                cwsp_sb[:, c, o, :],
                                g_t[:, h * HALF:(h + 1) * HALF],
                                start=False, stop=last,
                                skip_group_check=True,
                            )

            if loop_r:
                with tc.For_i(0, loop_r, 1,
                              hint_engines=(mybir.EngineType.Activation,
                                            mybir.EngineType.DVE,
                                            mybir.EngineType.PE)):
                    compute_body()
            else:
                compute_body()

            out_sb = singles.tile([OSH, B], F32)
            nc.scalar.activation(out_sb, psum_acc, AF.Identity,
                                 bias=bias_sb[:, 0:1], scale=1.0)
            nc.sync.dma_start(out_d[:], out_sb[:])

    nc.compile()
    return nc


def _plane(a):
    """[OSH, IN] param -> [128 part, NCH, OSH] per-partition plane."""
    return np.ascontiguousarray(
        a.reshape(OSH, NCH, 128).transpose(2, 1, 0).astype(np.float32))


def _host_prep(inp):
    x = inp["x"]
    xT = np.ascontiguousarray(x.T.reshape(NCH, 128, B).astype(np.float32))
    maps = []
    for k in range(NCORES):
        sl = slice(k * OSH, (k + 1) * OSH)
        fk = inp["frequency"][sl]
        sk = inp["scale"][sl]
        tk = inp["translation"][sl]
        cwk = inp["chirplet_weights"][sl]
        bwk = inp["base_weight"][sl]
        u2 = (fk / sk) * 65536.0
        v2 = (0.25 - fk * tk / sk) * 65536.0
        w = 1.0 / (math.sqrt(2.0) * sk)
        p = -tk / (math.sqrt(2.0) * sk)
        lv = _plane((math.sqrt(math.pi) / 2.0) * cwk)    # [128, NCH, OSH]
        cwsp = np.zeros((128, NCH, OSH, OSH), dtype=np.float32)
        cwsp[:, :, np.arange(OSH), np.arange(OSH)] = lv
        pf32 = np.ascontiguousarray(np.stack(
            [_plane(u2), _plane(v2), _plane(w), _plane(p), _plane(bwk)],
            axis=2))                                     # [128, NCH, 5, OSH]
        maps.append({
            "xT": xT,
            "pf32": pf32,
            "cwsp": cwsp.astype(ml_dtypes.bfloat16),
            "biasv": np.ascontiguousarray(
                inp["bias"][sl].reshape(OSH, 1).astype(np.float32)),
        })
    return maps


def kernel(**inputs):
    global _nc_cache, LAST_RESULT
    np_in = {k: np.asarray(v, dtype=np.float32) for k, v in inputs.items()}
    if _nc_cache is None:
        _nc_cache = _build_nc()
    in_maps = _host_prep(np_in)
    res = run_bass_kernel_spmd(
        _nc_cache, in_maps, core_ids=list(range(NCORES)), trace=TRACE)
    LAST_RESULT = res
    shards = [r["out"] for r in res.results]          # each [OSH, B]
    full = np.concatenate(shards, axis=0)             # [OUT, B]
    return np.ascontiguousarray(full.T)               # [B, OUT] fp32
